# revision 25
# baseline (speedup 1.0000x reference)
"""Self-contained Trainium2 (Bass) kernel for the 3-layer GCN
nn_FeaturePropagationModule problem: 100K nodes, 1.6M edges,
dims 512->64->128->40, log_softmax output, 8 NeuronCores.

Strategy (sharding_hint: shard nodes + edges by destination, replicate
weights): nodes are permuted into 8 shards x 98 blocks x 128 dsts
(degree-balanced); per layer each core computes y_own = (dis*h)@W for
its shard, AllGathers the bf16 y table, then aggregates its own dst
blocks. Aggregation uses per-edge dma_gather of 256B source rows from
the y table, batched into one big gather per (7-block group, 25088-row
src bucket) to amortize the ~1us SWDGE fixed cost; edge chunks of 128
are laid out contiguously per bucket stream (chunks may straddle two
adjacent dst blocks; a second one-hot plane handles the straddle), so
padding is only the cross-core max per (block,bucket) cell (~6%).
Scatter-add at the destinations runs on the tensor engine: one-hot
S = is_equal(dcol, iota) built on the vector engine in bf16 (2x rate),
then S^T @ msgs accumulated in PSUM per dst block. Self-loop terms
skip the gather entirely (identity matmul of the block's own y).
Epilogue fuses relu + symmetric norm into one activation
(relu(s*x) = s*relu(x) for s>0); final log_softmax is computed with a
batched exp per block plus a single ln over all blocks (avoids
activation-table thrash). Biases are zero per the problem spec
(nonzero-bias inputs fall back to a numpy path). Host un-permutes.
"""
import heapq
import numpy as np

import concourse.bacc as bacc
import concourse.mybir as mybir
from concourse.bass_utils import run_bass_kernel_spmd
from concourse.masks import make_identity
from concourse.tile import TileContext

FP = mybir.dt.float32
BF = mybir.dt.bfloat16
I16 = mybir.dt.int16
TW = 128  # gather-table width (bf16 -> 256B rows)
NCORES = 8
N_NODES = 100000
NB_BLOCKS = 98
GRP = 7  # dst blocks per gather group


# ---------------- host-side preprocessing ----------------


def preprocess(edge_index: np.ndarray, N: int, NB: int):
    SHARD = NB * 128
    PADN = NCORES * SHARD
    BUCKET = PADN // 4
    assert BUCKET < 32768 and N <= PADN
    src = edge_index[0].astype(np.int64)
    dst = edge_index[1].astype(np.int64)
    E = src.shape[0]

    deg = np.bincount(dst, minlength=N).astype(np.float64) + 1.0
    dis = (1.0 / np.sqrt(deg)).astype(np.float32)

    # degree-balanced assignment of nodes to (core, block) bins
    w = deg
    order = np.argsort(-w, kind="stable")
    nbins = NCORES * NB
    heap = [(0.0, b) for b in range(nbins)]
    heapq.heapify(heap)
    bin_nodes: list[list[int]] = [[] for _ in range(nbins)]
    for n_ in order:
        while True:
            s_, b = heapq.heappop(heap)
            if len(bin_nodes[b]) < 128:
                break
        bin_nodes[b].append(int(n_))
        if len(bin_nodes[b]) < 128:
            heapq.heappush(heap, (s_ + w[n_], b))
    perm_of_node = np.full(N, -1, dtype=np.int64)
    node_of_perm = np.full(PADN, -1, dtype=np.int64)
    for b in range(nbins):
        core, blk = divmod(b, NB)
        base = core * SHARD + blk * 128
        for i, n_ in enumerate(bin_nodes[b]):
            perm_of_node[n_] = base + i
            node_of_perm[base + i] = n_
    assert (perm_of_node >= 0).all()

    # real edges only; self-loops are added on-chip via an identity matmul
    psrc = perm_of_node[src]
    pdst = perm_of_node[dst]
    core_of = (pdst // SHARD).astype(np.int64)
    blk_of = ((pdst % SHARD) // 128).astype(np.int64)
    dcol_of = (pdst % 128).astype(np.int64)
    # y table rows are partition-major: node (c, b, p) -> row (c*128+p)*NB + b
    sc = psrc // SHARD
    sb = (psrc % SHARD) // 128
    sp = psrc % 128
    srow = (sc * 128 + sp) * NB + sb
    q_of = (srow // BUCKET).astype(np.int64)
    loc_of = (srow % BUCKET).astype(np.int64)

    # per (core, block, bucket) counts -> common slot schedule (max over cores)
    counts = np.zeros((NCORES, NB, 4), dtype=np.int64)
    np.add.at(counts, (core_of, blk_of, q_of), 1)
    cntmax = counts.max(axis=0)  # [NB, 4]
    assert (cntmax < 2 * 128 * 20).all()

    # slot layout: per bucket stream q, blocks packed back-to-back with
    # cntmax[b, q] slots each; streams rounded up to whole 128-edge chunks.
    P = np.zeros((NB, 4), dtype=np.int64)       # global slot start of (b, q)
    NCHq = np.zeros(4, dtype=np.int64)
    CH0 = np.zeros(4, dtype=np.int64)
    ch0 = 0
    for q in range(4):
        CH0[q] = ch0
        pos = ch0 * 128
        for b in range(NB):
            P[b, q] = pos
            pos += int(cntmax[b, q])
        NCHq[q] = -(-(pos - ch0 * 128) // 128)
        ch0 += int(NCHq[q])
    NCH = ch0
    NSLOT = NCH * 128

    # owner block of every slot (-1 for stream-tail padding)
    owner = np.full(NSLOT, -1, dtype=np.int64)
    for q in range(4):
        for b in range(NB):
            owner[P[b, q]:P[b, q] + int(cntmax[b, q])] = b
    blA = owner[np.arange(NCH) * 128]  # plane-A block of each chunk
    # tail chunks may start unowned; give them the previous owner so the
    # A-plane is well defined (their dcol stays 255 anyway)
    for ch in range(NCH):
        if blA[ch] < 0:
            blA[ch] = blA[ch - 1] if ch else 0

    # per-(b, q) compile-time schedule: column run in the merged one-hot
    # plane + the chunk ids it covers.  A chunk straddling blocks b-1 and b
    # appears twice in the plane: once with b-1's columns (A) and once with
    # b's (B) -- so each (b, q) needs exactly one is_equal + len(ids) matmuls.
    sched = []          # sched[b][q] = (plane_pos, [chunk ids])
    plane_chunks = []   # chunk id per plane column
    plane_useB = []     # whether that column uses the B (upper-block) values
    pos = 0
    for b in range(NB):
        row = []
        for q in range(4):
            s0 = int(P[b, q])
            s1 = s0 + int(cntmax[b, q]) - 1
            c0, c1 = s0 // 128, s1 // 128
            if blA[c0] == b:
                ids = list(range(c0, c1 + 1))
                useb = [False] * len(ids)
            else:
                assert blA[c0] == b - 1, (b, q, blA[c0])
                ids = list(range(c0, c1 + 1))
                useb = [True] + [False] * (len(ids) - 1)
            row.append((pos, ids))
            plane_chunks += ids
            plane_useB += useb
            pos += len(ids)
        sched.append(row)
    NCHM = pos
    plane_chunks = np.asarray(plane_chunks)
    plane_useB = np.asarray(plane_useB)

    # gather ranges per (group, bucket)
    grp = []
    for g in range(NB // GRP):
        row = []
        for q in range(4):
            b0, b1 = g * GRP, (g + 1) * GRP - 1
            gs = int(P[b0, q]) // 128
            ge = (int(P[b1, q]) + int(cntmax[b1, q]) - 1) // 128
            row.append((gs, ge))
        grp.append(row)
    NCHG = max(ge - gs + 1 for row in grp for (gs, ge) in row)

    # per-core slot fill: idx16 values + merged one-hot column plane
    idx16 = np.zeros((NCORES, 128, NCH * 8), dtype=np.int16)
    dcolM = np.full((NCORES, 128, NCHM), 255.0, dtype=np.float32)
    chunk_of = np.arange(NSLOT) // 128
    for c in range(NCORES):
        m = core_of == c
        eb, eq, ed, el = blk_of[m], q_of[m], dcol_of[m], loc_of[m]
        o = np.lexsort((el, eb, eq))
        eb, eq, ed, el = eb[o], eq[o], ed[o], el[o]
        # position within each (b, q) cell (cells appear sorted by (q, b))
        cell_key = eq * NB + eb
        first = np.r_[True, cell_key[1:] != cell_key[:-1]]
        cell_start = np.flatnonzero(first)
        offs = np.arange(eb.shape[0]) - np.repeat(
            cell_start, np.diff(np.r_[cell_start, eb.shape[0]]))
        slots = P[eb, eq] + offs
        v = np.zeros(NSLOT, dtype=np.int16)
        v[slots] = el.astype(np.int16)
        dA = np.full(NSLOT, 255.0, dtype=np.float32)
        dB = np.full(NSLOT, 255.0, dtype=np.float32)
        isA = blA[chunk_of[slots]] == eb
        assert (blA[chunk_of[slots]][~isA] == eb[~isA] - 1).all()
        dA[slots[isA]] = ed[isA]
        dB[slots[~isA]] = ed[~isA]
        cols = v.reshape(NCH, 8, 16).transpose(2, 0, 1).reshape(16, NCH * 8)
        idx16[c] = np.tile(cols, (8, 1))
        dA_rs = dA.reshape(NCH, 128).T
        dB_rs = dB.reshape(NCH, 128).T
        dcolM[c] = np.where(plane_useB[None, :], dB_rs[:, plane_chunks],
                            dA_rs[:, plane_chunks])
    # duplicate each column value into pairs so the DVE one-hot build can
    # use a packed (stride-1) last dim -> 2x perf mode
    dcolM = np.repeat(dcolM[..., None], 2, axis=-1)  # [NCORES, 128, NCHM, 2]

    dis_pad = np.zeros(PADN, dtype=np.float32)
    real = node_of_perm >= 0
    dis_pad[real] = dis[node_of_perm[real]]
    dis_cb = dis_pad.reshape(NCORES, NB, 128).transpose(0, 2, 1).copy()

    return dict(
        perm_of_node=perm_of_node, node_of_perm=node_of_perm, PADN=PADN,
        SHARD=SHARD, NB=NB, BUCKET=BUCKET, NCH=NCH, NCHG=NCHG, NCHM=NCHM,
        sched=sched, grp=grp, idx16=idx16, dcolM=dcolM,
        dis=dis, dis_cb=dis_cb,
    )


def make_inputs(x, W1, W2, W3, pp, KIN):
    """Build per-core in_maps (list of dicts) for the bass kernel."""
    import ml_dtypes
    F1, F2 = W1.shape[1], W2.shape[1]
    COUT = W3.shape[1]
    SHARD, PADN, NB = pp["SHARD"], pp["PADN"], pp["NB"]
    perm = pp["perm_of_node"]
    KK = KIN // 128
    bf = ml_dtypes.bfloat16

    xs = x.astype(np.float32) * pp["dis"][:, None]
    xs_p = np.zeros((PADN, KIN), np.float32)
    xs_p[perm] = xs
    iota = np.tile(np.arange(128, dtype=np.float32), (128, 1))
    w1p = np.zeros((KIN, TW), np.float32); w1p[:, :F1] = W1
    w1r = w1p.reshape(KK, 128, TW).transpose(1, 0, 2).reshape(128, KK * TW)
    w2p = np.zeros((F1, TW), np.float32); w2p[:, :F2] = W2
    w3p = np.zeros((F2, TW), np.float32); w3p[:, :COUT] = W3
    in_maps = []
    for c in range(NCORES):
        blk = xs_p[c * SHARD:(c + 1) * SHARD, :].reshape(NB, 128, KK, 128)
        # [b, node c, kk, feat p] -> [b, p, kk, c]; per-partition contiguous 1KB
        xsb = np.ascontiguousarray(blk.transpose(0, 3, 2, 1)).astype(bf)
        in_maps.append({
            "xsb": xsb.reshape(NB, 128, KK * 128),
            "w1": np.ascontiguousarray(w1r).astype(bf),
            "w2": np.ascontiguousarray(w2p).astype(bf),
            "w3": np.ascontiguousarray(w3p).astype(bf),
            "dcolM": np.ascontiguousarray(pp["dcolM"][c]).astype(bf),
            "idx": np.ascontiguousarray(pp["idx16"][c]),
            "iota": iota.astype(bf),
            "dis": np.ascontiguousarray(pp["dis_cb"][c]),
            "dis2": np.ascontiguousarray(pp["dis_cb"][c] ** 2),
        })
    return in_maps


# ---------------- bass program builder ----------------


def build(params):
    NB = params["NB"]; NCH = params["NCH"]; NCHG = params["NCHG"]
    NCHM = params["NCHM"]
    sched = params["sched"]; grp = params["grp"]
    BUCKET = params["BUCKET"]; PADN = params["PADN"]
    KIN = params["KIN"]; F1 = params["F1"]; F2 = params["F2"]
    F3 = params["F3"]; COUT = params["COUT"]
    REPEAT = params.get("repeat", 1)
    TIMING_LOOP = params.get("timing_loop", 0)
    MOCK_CC = params.get("mock_collectives", False)
    SHARD = NB * 128
    KK = KIN // 128
    NGRP = NB // GRP
    KM_MAX = max(len(ids) for row in sched for (_, ids) in row)

    nc = bacc.Bacc(num_swdge_queues=4)
    xsb = nc.declare_dram_parameter("xsb", [NB, 128, KK * 128], BF, isOutput=False)
    w1 = nc.declare_dram_parameter("w1", [128, KK * TW], BF, isOutput=False)
    w2 = nc.declare_dram_parameter("w2", [F1, TW], BF, isOutput=False)
    w3 = nc.declare_dram_parameter("w3", [F2, TW], BF, isOutput=False)
    dcolM_in = nc.declare_dram_parameter("dcolM", [128, NCHM, 2], BF, isOutput=False)
    idx_in = nc.declare_dram_parameter("idx", [128, NCH * 8], I16, isOutput=False)
    iota_in = nc.declare_dram_parameter("iota", [128, 128], BF, isOutput=False)
    dis_in = nc.declare_dram_parameter("dis", [128, NB], FP, isOutput=False)
    dis2_in = nc.declare_dram_parameter("dis2", [128, NB], FP, isOutput=False)
    out_ext = nc.declare_dram_parameter("out", [SHARD, COUT], FP, isOutput=True)

    # y tables are partition-major: node (c, b, p) lives at row (c*128+p)*NB+b
    y_own = [nc.dram_tensor(f"y{l}_own", [128, NB * TW], BF) for l in (1, 2, 3)]
    y_full = [nc.dram_tensor(f"y{l}_full", [NCORES * 128, NB * TW], BF,
                             addr_space="Shared")
              for l in (1, 2, 3)]
    rg = [list(range(NCORES))]

    with TileContext(nc) as tc:
        with tc.tile_pool(name="const", bufs=1) as cpool, \
             tc.tile_pool(name="xb", bufs=4) as xpool, \
             tc.tile_pool(name="ybuf", bufs=1) as ybpool, \
             tc.tile_pool(name="gt", bufs=1) as gtpool, \
             tc.tile_pool(name="msg", bufs=7) as mpool, \
             tc.tile_pool(name="sa", bufs=5) as sapool, \
             tc.tile_pool(name="fin", bufs=3) as fpool, \
             tc.tile_pool(name="ps", bufs=2, space="PSUM") as pspool, \
             tc.tile_pool(name="pagg", bufs=4, space="PSUM") as papool, \
             tc.tile_pool(name="ptr", bufs=2, space="PSUM") as ptpool:

            ident = cpool.tile([128, 128], BF)
            make_identity(nc, ident[:])
            iota = cpool.tile([128, 128], BF)
            nc.sync.dma_start(out=iota[:], in_=iota_in[:])
            dcolM = cpool.tile([128, NCHM, 2], BF)
            nc.sync.dma_start(out=dcolM[:], in_=dcolM_in[:])
            idxsb = cpool.tile([128, NCH * 8], I16)
            nc.sync.dma_start(out=idxsb[:], in_=idx_in[:])
            dis = cpool.tile([128, NB], FP)
            nc.sync.dma_start(out=dis[:], in_=dis_in[:])
            dis2 = cpool.tile([128, NB], FP)
            nc.sync.dma_start(out=dis2[:], in_=dis2_in[:])
            w1sb = cpool.tile([128, KK * TW], BF)
            nc.sync.dma_start(out=w1sb[:], in_=w1[:])
            w2sb = cpool.tile([F1, TW], BF)
            nc.sync.dma_start(out=w2sb[:], in_=w2[:])
            w3sb = cpool.tile([F2, TW], BF)
            nc.sync.dma_start(out=w3sb[:], in_=w3[:])

            gT = gtpool.tile([128, SHARD], BF, tag="gT")
            ybuf = ybpool.tile([128, NB, TW], BF, tag="ybuf")
            zall = ybpool.tile([128, NB, COUT], FP, tag="zall")
            nmall = ybpool.tile([128, NB], FP, tag="nmall")
            sall = ybpool.tile([128, NB], FP, tag="sall")
            lgall = ybpool.tile([128, NB], FP, tag="lgall")
            bball = ybpool.tile([128, NB], FP, tag="bball")

            def allgather(l):
                if MOCK_CC:
                    for s in range(NCORES):
                        nc.sync.dma_start(
                            out=y_full[l][s * 128:(s + 1) * 128, :],
                            in_=y_own[l][:])
                else:
                    nc.gpsimd.collective_compute(
                        "AllGather", mybir.AluOpType.bypass, replica_groups=rg,
                        ins=[y_own[l][:]], outs=[y_full[l][:]])

            def y_write(l):
                nc.sync.dma_start(
                    out=y_own[l][:],
                    in_=ybuf[:].rearrange("p b f -> p (b f)"))

            def one_hot(out4, c0, k):
                # out[p, j, c] = (dcolM[p, c0+j] == c); all last dims packed
                # (pair-duplicated dcol) so DVE runs in 2x mode
                nc.vector.tensor_tensor(
                    out=out4,
                    in0=dcolM[:, c0:c0 + k, :].unsqueeze(2)
                        .to_broadcast([128, k, 64, 2]),
                    in1=iota[:].rearrange("p (a b) -> p a b", b=2)
                        .unsqueeze(1).to_broadcast([128, k, 64, 2]),
                    op=mybir.AluOpType.is_equal,
                )

            def agg_phase(l, F, last):
                yf = y_full[l]
                for g in range(NGRP):
                    msgs = []
                    subcalls = []
                    for q in range(4):
                        gs, ge = grp[g][q]
                        nch = ge - gs + 1
                        mt = mpool.tile([128, NCHG, TW], BF, tag="msg")
                        # the gather ucode handles at most 1024 indices
                        # (8 chunks) per call; split the range evenly
                        nsub = -(-nch // 8)
                        o = 0
                        for i in range(nsub):
                            sub = nch // nsub + (1 if i < nch % nsub else 0)
                            subcalls.append((o, q, mt, gs, sub))
                            o += sub
                        msgs.append((mt, gs))
                    # round-robin the sub-calls across the 4 queues so the
                    # Q7 generates into one ring while another drains
                    subcalls.sort(key=lambda t: (t[0], t[1]))
                    for (o, q, mt, gs, sub) in subcalls:
                        nc.gpsimd.dma_gather(
                            mt[:, o:o + sub, :],
                            yf[q * 256:(q + 1) * 256, :]
                                .rearrange("r (b f) -> (r b) f", f=TW),
                            idxsb[:, (gs + o) * 8:(gs + o + sub) * 8],
                            sub * 128, sub * 128, TW, queue_num=q,
                        )
                    for b in range(g * GRP, (g + 1) * GRP):
                        pa = papool.tile([128, F], FP, tag="pa")
                        done = 0
                        for q in range(4):
                            pos, ids = sched[b][q]
                            mt, gs = msgs[q]
                            k2 = len(ids)
                            SA = sapool.tile([128, KM_MAX, 128], BF, tag="sa")
                            one_hot(SA[:, :k2, :].rearrange(
                                "p k (a b) -> p k a b", b=2), pos, k2)
                            for j, ch in enumerate(ids):
                                nc.tensor.matmul(
                                    pa[:], SA[:, j, :], mt[:, ch - gs, :F],
                                    start=(done == 0), stop=False)
                                done += 1
                        # self-loop: pa += I^T @ y_own_block
                        nc.tensor.matmul(
                            pa[:], ident[:], ybuf[:, b, :F],
                            start=False, stop=True)
                        if not last:
                            gact = fpool.tile([128, F], BF, tag="g")
                            nc.scalar.activation(
                                gact[:], pa[:], mybir.ActivationFunctionType.Relu,
                                scale=dis2[:, b:b + 1])
                            pt = ptpool.tile([F, 128], BF, tag="pt")
                            nc.tensor.transpose(
                                out=pt[:], in_=gact[:], identity=ident[:])
                            nc.scalar.activation(
                                gT[:F, b * 128:(b + 1) * 128], pt[:],
                                mybir.ActivationFunctionType.Copy)
                        else:
                            nc.vector.tensor_scalar(
                                out=zall[:, b, :], in0=pa[:, :COUT],
                                scalar1=dis[:, b:b + 1], scalar2=None,
                                op0=mybir.AluOpType.mult)
                            nc.vector.tensor_reduce(
                                nmall[:, b:b + 1], zall[:, b, :],
                                mybir.AxisListType.X,
                                mybir.AluOpType.max, negate=True)
                            e = fpool.tile([128, COUT], FP, tag="e")
                            nc.scalar.activation(
                                e[:], zall[:, b, :],
                                mybir.ActivationFunctionType.Exp,
                                bias=nmall[:, b:b + 1])
                            nc.vector.tensor_reduce(
                                sall[:, b:b + 1], e[:], mybir.AxisListType.X,
                                mybir.AluOpType.add)
                if last:
                    nc.scalar.activation(
                        lgall[:], sall[:], mybir.ActivationFunctionType.Ln)
                    nc.vector.tensor_tensor(
                        out=bball[:], in0=nmall[:], in1=lgall[:],
                        op=mybir.AluOpType.subtract)
                    nc.vector.tensor_tensor(
                        out=zall[:], in0=zall[:],
                        in1=bball[:].unsqueeze(2).to_broadcast([128, NB, COUT]),
                        op=mybir.AluOpType.add)
                    nc.sync.dma_start(
                        out=out_ext[:].rearrange("(c p) f -> p c f", p=128),
                        in_=zall[:])

            def pipeline(with_ag):
                # ---- L1 y: stream x blocks, accumulate KK chunks in PSUM ----
                for r in range(NB):
                    xb = xpool.tile([128, KK * 128], BF, tag="xb")
                    nc.sync.dma_start(out=xb[:], in_=xsb[r])
                    ps = pspool.tile([128, TW], FP, tag="psy")
                    for kk in range(KK):
                        nc.tensor.matmul(
                            ps[:], xb[:, kk * 128:(kk + 1) * 128],
                            w1sb[:, kk * TW:(kk + 1) * TW],
                            start=(kk == 0), stop=(kk == KK - 1))
                    nc.scalar.activation(
                        ybuf[:, r, :], ps[:],
                        mybir.ActivationFunctionType.Copy)
                y_write(0)
                if with_ag: allgather(0)
                agg_phase(0, F1, last=False)

                # ---- L2 y ----
                for r in range(NB):
                    ps = pspool.tile([128, TW], FP, tag="psy")
                    nc.tensor.matmul(
                        ps[:], gT[:F1, r * 128:(r + 1) * 128], w2sb[:],
                        start=True, stop=True)
                    nc.scalar.activation(
                        ybuf[:, r, :], ps[:],
                        mybir.ActivationFunctionType.Copy)
                y_write(1)
                if with_ag: allgather(1)
                agg_phase(1, F2, last=False)

                # ---- L3 y ----
                for r in range(NB):
                    ps = pspool.tile([128, TW], FP, tag="psy")
                    nc.tensor.matmul(
                        ps[:], gT[:F2, r * 128:(r + 1) * 128], w3sb[:],
                        start=True, stop=True)
                    nc.scalar.activation(
                        ybuf[:, r, :], ps[:],
                        mybir.ActivationFunctionType.Copy)
                y_write(2)
                if with_ag: allgather(2)
                agg_phase(2, F3, last=True)

            for _rep in range(REPEAT):
                pipeline(True)
            if TIMING_LOOP:
                with tc.For_i(0, TIMING_LOOP, 1) as _:
                    pipeline(False)

    nc.compile()
    return nc


def build_params(pp, KIN, F1, F2, COUT, **extra):
    return dict(NB=pp["NB"], NCH=pp["NCH"], NCHG=pp["NCHG"],
                NCHM=pp["NCHM"],
                sched=pp["sched"], grp=pp["grp"], BUCKET=pp["BUCKET"],
                PADN=pp["PADN"], KIN=KIN, F1=F1, F2=F2,
                F3=64 if COUT <= 64 else 128, COUT=COUT, **extra)


_CACHE = {}


def _reference_numpy(x, edge_index, W1, b1, W2, b2, W3, b3):
    src = edge_index[0].astype(np.int64); dst = edge_index[1].astype(np.int64)
    N = x.shape[0]
    deg = np.bincount(dst, minlength=N) + 1.0
    dis = 1.0 / np.sqrt(deg)
    norm = (dis[src] * dis[dst]).astype(np.float32)

    def layer(xv, W, b):
        xw = xv @ W
        agg = np.zeros_like(xw)
        np.add.at(agg, dst, xw[src] * norm[:, None])
        agg += xw * (dis * dis)[:, None].astype(np.float32)
        return agg + b

    h1 = np.maximum(layer(x.astype(np.float32), W1, b1), 0)
    h2 = np.maximum(layer(h1, W2, b2), 0)
    z = layer(h2, W3, b3)
    m = z.max(axis=1, keepdims=True)
    return (z - m - np.log(np.exp(z - m).sum(axis=1, keepdims=True))).astype(np.float32)


def kernel(x, edge_index, W1, b1, W2, b2, W3, b3):
    x = np.asarray(x); edge_index = np.asarray(edge_index)
    W1 = np.asarray(W1, np.float32); W2 = np.asarray(W2, np.float32)
    W3 = np.asarray(W3, np.float32)
    b1 = np.asarray(b1, np.float32); b2 = np.asarray(b2, np.float32)
    b3 = np.asarray(b3, np.float32)
    if np.any(b1) or np.any(b2) or np.any(b3):
        # device kernel fuses the (spec-guaranteed zero) biases away
        return _reference_numpy(x, edge_index, W1, b1, W2, b2, W3, b3)

    KIN = x.shape[1]
    F1, F2 = W1.shape[1], W2.shape[1]
    COUT = W3.shape[1]
    pp = preprocess(edge_index, x.shape[0], NB_BLOCKS)
    in_maps = make_inputs(x, W1, W2, W3, pp, KIN)
    key = ("nc", pp["NCH"], pp["NCHM"], pp["NCHG"])
    if key not in _CACHE:
        _CACHE[key] = build(build_params(pp, KIN, F1, F2, COUT))
    nc = _CACHE[key]
    res = run_bass_kernel_spmd(nc, in_maps, list(range(NCORES)))
    full = np.concatenate([res.results[c]["out"] for c in range(NCORES)], axis=0)
    return np.ascontiguousarray(full[pp["perm_of_node"]]).astype(np.float32)
